# revision 33
# baseline (speedup 1.0000x reference)
"""Trainium2 Bass kernel for nn_CNN3_FPB (dense CNN + bypass MLP + FC head).

Data-parallel over 8 NeuronCores: batch 2048 -> 256 per core. All weights
replicated. Inside each core:

  warmup: dummy matmuls on zeroed scratch bridge the initial x-DMA wait and
          get the PE HAM clock-gate to 8/8 before real work starts.
  stage0: y[p,b,c] = Wp1 @ xT[:,b,c] + Wp0 @ x0[:,b] + bp, relu
          (K=128 matmul, M=64 -> column-tiled: two concurrent M=64 matmuls
          in array col-halves). Writes h1d rows 0:64; rows 64:128 (the
          shift-by-one copy that lets conv1 taps pack) are filled by one
          SBUF->SBUF DMA per block on the gpsimd queue.
  conv1:  K=3 stride 1: 2 matmuls/chunk: taps(0,1) as one K=128 matmul on
          h1d, tap2 as K=64 matmul on h1d rows 64-127.
  conv2:  K=3 stride 2, Cin=128, Cout=256 (2 M-chunks), 3 K=128 matmuls
  conv3:  K=3 stride 2, Cin=256 (2 K-chunks), Cout=256 (2 M-chunks)

  The block loop is software-pipelined: iteration b emits
      stage0(b), conv2(b-1), conv3(b-1), conv1(b)
  so the ~10us of conv2/conv3 PE work hides stage0(b)'s epilogue + h1d-copy
  latency and the PE never stalls (stalls re-throttle the HAM clock to 1.2GHz,
  which is what made the naive ordering slow).

  fc1:    f-outer: per 128-wide f-chunk, stream its weights in 4 quarter
          chunks (gpsimd DMA queue, first 4 prefetched during the trunk),
          run 130 accumulating K=128 matmuls into one PSUM bank, relu, and
          accumulate fc2 inline in a second bank. No serial tail.

Activation layout: [channels(part), position, batch]; stride-2 conv inputs
are parity-split (even/odd position tensors). All trunk matmuls bf16
(fp8 measured >3% output error on this net - out of tolerance).
"""

import os
import sys
from contextlib import ExitStack

import numpy as np

for _p in ("/opt/trn_rl_repo", "/root/.axon_site/_ro/trn_rl_repo"):
    if os.path.isdir(_p) and _p not in sys.path:
        sys.path.insert(0, _p)

import ml_dtypes  # noqa: E402
import concourse.bass as bass  # noqa: E402
from concourse import bacc  # noqa: E402
import concourse.mybir as mybir  # noqa: E402
import concourse.tile as tile  # noqa: E402

F32 = mybir.dt.float32
F32R = mybir.dt.float32r
BF16 = mybir.dt.bfloat16
RELU = mybir.ActivationFunctionType.Relu
ADD = mybir.AluOpType.add
MAX = mybir.AluOpType.max

# Problem constants (hardcoded; must match the grading problem).
B, CL, IL = 2048, 256, 64
NCORES = 8
BC = B // NCORES  # 256 samples per core
BB = 16           # samples per conv block
NBLK = BC // BB
PC = 64
CH1, CH2, CH3 = 128, 256, 256
L1, L2, L3 = 255, 128, 64
F1 = 1024
OUTC = 2

NBIAS = 19   # bias columns: see _prep_bias
NWARM = 26   # dummy warmup matmuls (N=256 each) before first real work
NWARM2 = 10  # filler matmuls between stage0(0) and conv1(0)

# fc1 weight streaming: 8 chunks per f-chunk (65 slabs = 64 l3 + bypass)
NQ = 8
QS = 9
NSQ = [9, 8, 8, 8, 8, 8, 8, 8]
OFF = [0, 9, 17, 25, 33, 41, 49, 57]
NGATE = 12  # chunks prefetched during the trunk (gated, scalar queue)


def build_nc():
    nc = bacc.Bacc()
    TDT = BF16

    xs = nc.declare_dram_parameter("xs", [NBLK, 128, CL, BB], TDT, isOutput=False)
    x0s = nc.declare_dram_parameter("x0s", [64, BC], F32R, isOutput=False)
    wstk = nc.declare_dram_parameter("wstk", [128, 64], TDT, isOutput=False)
    # w1n: [:,0:128] = [W1tap0.T ; W1tap1.T] stacked; [64:,128:256] = W1tap2.T
    w1n = nc.declare_dram_parameter("w1n", [128, 256], TDT, isOutput=False)
    w2 = nc.declare_dram_parameter("w2", [128, 3, CH2], TDT, isOutput=False)
    w3 = nc.declare_dram_parameter("w3", [128, 2, 3, CH3], TDT, isOutput=False)
    wb1 = nc.declare_dram_parameter("wb1", [64, 64], F32R, isOutput=False)
    wb2 = nc.declare_dram_parameter("wb2", [64, 128], F32R, isOutput=False)
    wb3 = nc.declare_dram_parameter("wb3", [128, 256], F32R, isOutput=False)
    # fc1 weights, f-chunk-major chunks:
    # [f, chunk, 128 part, <=9 slabs, 2 ci, 128 fcols]
    # slab g = OFF[chunk]+s: g<64 -> l3 position g, g==64 -> bypass
    wgf = nc.declare_dram_parameter("wgf", [8, NQ, 128, QS, 2, 128], BF16,
                                    isOutput=False)
    wfc2 = nc.declare_dram_parameter("wfc2", [128, 8, OUTC], F32R, isOutput=False)
    bias = nc.declare_dram_parameter("bias", [128, NBIAS], F32, isOutput=False)
    out = nc.declare_dram_parameter("out", [OUTC, BC], F32, isOutput=True)

    with ExitStack() as ctx:
        tc = ctx.enter_context(tile.TileContext(nc))
        wpool = ctx.enter_context(tc.tile_pool(name="wpool", bufs=1))
        xpool = ctx.enter_context(tc.tile_pool(name="xpool", bufs=2))
        wgpool = ctx.enter_context(tc.tile_pool(name="wgpool", bufs=NGATE))
        h1pool = ctx.enter_context(tc.tile_pool(name="h1pool", bufs=2))
        h2pool = ctx.enter_context(tc.tile_pool(name="h2pool", bufs=2))
        h3pool = ctx.enter_context(tc.tile_pool(name="h3pool", bufs=2))
        zpool = ctx.enter_context(tc.tile_pool(name="zpool", bufs=1))
        z2pool = ctx.enter_context(tc.tile_pool(name="z2pool", bufs=2))
        spool = ctx.enter_context(tc.tile_pool(name="spool", bufs=1))

        # ---- warmup scratch (no DMA deps): keep PE busy from t~0 ----
        scratch = wpool.tile([128, 320], TDT)
        nc.gpsimd.memset(scratch[:], 0.0)

        # ---- startup DMAs distributed across the 3 DMA-capable queues
        # (sync/scalar/gpsimd; each sustains only ~105GB/s, so the first two
        # x blocks are 3-way split and weights spread over the queues) ----
        xt_pre = {}
        wstk_t = wpool.tile([128, 64], TDT)
        nc.scalar.dma_start(wstk_t[:], wstk[:])
        x0_t = wpool.tile([64, BC], F32R)
        nc.scalar.dma_start(x0_t[:], x0s[:])
        xt0 = xpool.tile([128, CL, BB], TDT, name="xt0", tag="xt")
        nc.sync.dma_start(xt0[:, 0:86, :], xs[0, :, 0:86, :])
        nc.gpsimd.dma_start(xt0[:, 171:256, :], xs[0, :, 171:256, :])
        xt_pre[0] = xt0
        wb1_t = wpool.tile([64, 64], F32R)
        nc.scalar.dma_start(wb1_t[:], wb1[:])
        wb2_t = wpool.tile([64, 128], F32R)
        nc.scalar.dma_start(wb2_t[:], wb2[:])
        w1_t = wpool.tile([128, 256], TDT)
        nc.gpsimd.dma_start(w1_t[:], w1n[:])
        xt1 = xpool.tile([128, CL, BB], TDT, name="xt1", tag="xt")
        nc.sync.dma_start(xt1[:, 0:86, :], xs[1, :, 0:86, :])
        wb3_t = wpool.tile([128, 256], F32R)
        nc.scalar.dma_start(wb3_t[:], wb3[:])
        bias_t = wpool.tile([128, NBIAS], F32)
        nc.scalar.dma_start(bias_t[:], bias[:])
        nc.scalar.dma_start(xt0[:, 86:171, :], xs[0, :, 86:171, :])
        nc.gpsimd.dma_start(xt1[:, 171:256, :], xs[1, :, 171:256, :])
        nc.scalar.dma_start(xt1[:, 86:171, :], xs[1, :, 86:171, :])
        xt_pre[1] = xt1
        w2_t = wpool.tile([128, 3, CH2], TDT)
        nc.gpsimd.dma_start(w2_t[:], w2[:])
        w3_t = wpool.tile([128, 2, 3, CH3], TDT)
        nc.scalar.dma_start(w3_t[:], w3[:])
        wfc2_t = wpool.tile([128, 8, OUTC], F32R)
        nc.scalar.dma_start(wfc2_t[:], wfc2[:])

        bp_lo = bias_t[:64, 0:1]
        bp_hi = bias_t[64:128, 0:1]
        b1_ap = bias_t[:, 1:2]

        cpsum_ctx = ExitStack()
        cpsum = cpsum_ctx.enter_context(tc.tile_pool(name="cpsum", bufs=8, space="PSUM"))

        # ---- warmup: one long accumulation chain of zero matmuls ----
        wps = cpsum.tile([128, 512], F32, tag="ps")
        for i in range(NWARM):
            nc.tensor.matmul(
                wps[:64, 0:256], scratch[:, 0:64], scratch[:, 64:320],
                start=(i == 0), stop=(i == NWARM - 1),
            )

        def bypass_mlp():
            # tiny MLP on x0; emitted after stage0(0) so it fills the PE
            # while block 0's h1d copy completes
            ps = cpsum.tile([64, BC], F32, tag="ps")
            nc.tensor.matmul(ps[:], wb1_t[:], x0_t[:], start=True, stop=True)
            s1 = spool.tile([64, BC], F32R)
            nc.scalar.activation(s1[:], ps[:], RELU, bias=bias_t[:64, 6:7])
            ps = cpsum.tile([128, BC], F32, tag="ps")
            nc.tensor.matmul(ps[:], wb2_t[:], s1[:], start=True, stop=True)
            s2 = spool.tile([128, BC], F32R)
            nc.scalar.activation(s2[:], ps[:], RELU, bias=bias_t[:, 7:8])
            fbyp = spool.tile([128, 2, BC], BF16)
            for m in range(2):
                ps = cpsum.tile([128, BC], F32, tag="ps")
                nc.tensor.matmul(
                    ps[:], wb3_t[:, m * 128 : (m + 1) * 128], s2[:],
                    start=True, stop=True,
                )
                nc.vector.tensor_scalar(
                    fbyp[:, m, :], ps[:], bias_t[:, 8 + m : 9 + m], 0.0, ADD, MAX
                )
            return fbyp

        # ---- resident conv3 output (fc1 rhs), bf16: [ci, cich, l3, b] ----
        zres = zpool.tile([128, 2, L3, BC], BF16)

        S0_CHUNKS = [(1 + 32 * j, 32 if j < 7 else 31) for j in range(8)]
        C1_CHUNKS = [(32 * j, 32 if j < 7 else 31) for j in range(8)]

        def stage0(blk):
            if blk in xt_pre:
                xt = xt_pre[blk]
            else:
                xt = xpool.tile([128, CL, BB], TDT, name="xt", tag="xt")
                nc.sync.dma_start(xt[:], xs[blk, :, :, :])

            # stage0 -> h1d [128, 256, BB]:
            #   rows 0:64,  col j = y[pos j-1]  (j=1..255; j=0 zero pad)
            #   rows 64:128 col j = y[pos j]    (j=0..254; j=255 zero pad)
            h1d = h1pool.tile([128, 256, BB], TDT)
            nc.gpsimd.memset(h1d[0:64, 0:1, :], 0.0)
            nc.gpsimd.memset(h1d[64:128, 255:256, :], 0.0)
            for q in range(4):
                ps = cpsum.tile([128, 512], F32, tag="ps")
                for half in range(2):
                    c0, cc = S0_CHUNKS[2 * q + half]
                    nc.tensor.matmul(
                        ps[64 * half : 64 * half + 64, 0 : cc * BB],
                        wstk_t[:],
                        xt[:, c0 : c0 + cc, :].rearrange("p c b -> p (c b)"),
                        start=True, stop=True,
                    )
                for half in range(2):
                    c0, cc = S0_CHUNKS[2 * q + half]
                    src = ps[64 * half : 64 * half + 64, 0 : cc * BB]
                    bsl = bp_hi if half else bp_lo
                    dst = h1d[0:64, c0 : c0 + cc, :].rearrange("p c b -> p (c b)")
                    if half == 0:
                        nc.scalar.activation(dst, src, RELU, bias=bsl)
                    else:
                        nc.vector.tensor_scalar(dst, src, bsl, 0.0, ADD, MAX)
            # shift-by-one copy for conv1 tap packing (DMA engine)
            nc.gpsimd.dma_start(h1d[64:128, 0:255, :], h1d[0:64, 1:256, :])
            return h1d

        def conv1(h1d):
            # conv1 -> h2 parity-split: h2e [128,128,BB] (pos 0,2,..254),
            # h2o [128,129,BB] (j=(pos+1)/2 for odd pos -1..255; pads j=0,128)
            h2e = h2pool.tile([128, 128, BB], TDT)
            h2o = h2pool.tile([128, 129, BB], TDT)
            nc.gpsimd.memset(h2o[:, 0:1, :], 0.0)
            nc.gpsimd.memset(h2o[:, 128:129, :], 0.0)
            for ci, (l0, lc) in enumerate(C1_CHUNKS):
                ps = cpsum.tile([128, 512], F32, tag="ps")
                nc.tensor.matmul(
                    ps[:, 0 : lc * BB], w1_t[:, 0:128],
                    h1d[:, l0 : l0 + lc, :].rearrange("p l b -> p (l b)"),
                    start=True, stop=False,
                )
                # tap2: full K=128 matmul; weight rows 0-63 are zero so the
                # h1d top half contributes nothing (keeps LDW double-buffered)
                nc.tensor.matmul(
                    ps[:, 0 : lc * BB], w1_t[:, 128:256],
                    h1d[:, l0 + 1 : l0 + 1 + lc, :].rearrange("p l b -> p (l b)"),
                    start=False, stop=True,
                )
                ps3 = ps.rearrange("p (t x) -> p t x", x=32)
                ne, no = (lc + 1) // 2, lc // 2
                nc.vector.tensor_scalar(
                    h2e[:, l0 // 2 : l0 // 2 + ne, :], ps3[:, :ne, 0:16],
                    b1_ap, 0.0, ADD, MAX,
                )
                nc.scalar.activation(
                    h2o[:, l0 // 2 + 1 : l0 // 2 + 1 + no, :], ps3[:, :no, 16:32],
                    RELU, bias=b1_ap,
                )
            return h2e, h2o

        def conv2(h2e, h2o):
            # conv2 -> h3 parity-split per ci-chunk: h3e [128,2,64,BB],
            # h3o [128,2,65,BB] (j=(pos+1)/2 for odd pos -1..127; pad j=0)
            h3e = h3pool.tile([128, 2, 64, BB], TDT)
            h3o = h3pool.tile([128, 2, 65, BB], TDT)
            nc.gpsimd.memset(h3o[:, :, 0:1, :], 0.0)
            for m in range(2):
                for pair in range(2):
                    for i in range(2):
                        ps = cpsum.tile([128, 512], F32, tag="ps")
                        l20 = 64 * pair + 32 * i
                        for k in range(3):
                            if k == 0:
                                rhs = h2o[:, l20 : l20 + 32, :]
                            elif k == 1:
                                rhs = h2e[:, l20 : l20 + 32, :]
                            else:
                                rhs = h2o[:, l20 + 1 : l20 + 33, :]
                            nc.tensor.matmul(
                                ps[:],
                                w2_t[:, k, m * 128 : (m + 1) * 128],
                                rhs.rearrange("p l b -> p (l b)"),
                                start=(k == 0), stop=(k == 2),
                            )
                        j0 = 32 * pair + 16 * i
                        ps3 = ps.rearrange("p (t x) -> p t x", x=32)
                        nc.scalar.activation(
                            h3e[:, m, j0 : j0 + 16, :], ps3[:, :, 0:16],
                            RELU, bias=bias_t[:, 2 + m : 3 + m],
                        )
                        nc.vector.tensor_scalar(
                            h3o[:, m, j0 + 1 : j0 + 17, :],
                            ps3[:, :, 16:32], bias_t[:, 2 + m : 3 + m], 0.0, ADD, MAX,
                        )
            return h3e, h3o

        def conv3(h3e, h3o, blk):
            b0 = blk * BB
            for m in range(2):
                for q in range(2):
                    ps = cpsum.tile([128, 512], F32, tag="ps")
                    l30 = 32 * q
                    acc = 0
                    for c in range(2):
                        for k in range(3):
                            if k == 0:
                                rhs = h3o[:, c, l30 : l30 + 32, :]
                            elif k == 1:
                                rhs = h3e[:, c, l30 : l30 + 32, :]
                            else:
                                rhs = h3o[:, c, l30 + 1 : l30 + 33, :]
                            nc.tensor.matmul(
                                ps[:],
                                w3_t[:, c, k, m * 128 : (m + 1) * 128],
                                rhs.rearrange("p l b -> p (l b)"),
                                start=(acc == 0), stop=(acc == 5),
                            )
                            acc += 1
                    ps3 = ps.rearrange("p (l b) -> p l b", b=BB)
                    dst = zres[:, m, l30 : l30 + 32, b0 : b0 + BB]
                    if (m + q) % 2 == 0:
                        nc.scalar.activation(
                            dst, ps3[:], RELU, bias=bias_t[:, 4 + m : 5 + m],
                        )
                    else:
                        nc.vector.tensor_scalar(
                            dst, ps3[:], bias_t[:, 4 + m : 5 + m], 0.0, ADD, MAX,
                        )

        # ---- software-pipelined trunk ----
        # fc1 weight chunks j=0..NGATE-1 prefetch during the trunk: a
        # 1-element vector "gate" write into the tile pins each DMA's start
        # to real-time block pace (the scheduler cannot hoist it into the
        # startup window); transfers go on the scalar queue, which carries
        # no other DMAs mid-trunk (gpsimd carries the h1d copies).
        fbyp = bypass_mlp()
        wg_tiles = {}
        prev = None
        for blk in range(NBLK):
            h1d = stage0(blk)
            if blk == 0:
                wps2 = cpsum.tile([128, 512], F32, tag="ps")
                for i in range(NWARM2):
                    nc.tensor.matmul(
                        wps2[:64, 0:256], scratch[:, 0:64], scratch[:, 64:320],
                        start=(i == 0), stop=(i == NWARM2 - 1),
                    )
            if prev is not None:
                h2e, h2o, pblk = prev
                h3e, h3o = conv2(h2e, h2o)
                conv3(h3e, h3o, pblk)
            if 3 <= blk < 3 + NGATE:
                j = blk - 3
                ns = NSQ[j % NQ]
                wt = wgpool.tile([128, QS, 2, 128], BF16, name="wq", tag="wgf")
                nc.vector.tensor_scalar_add(
                    wt[0:1, 0, 0, 0:1], bias_t[0:1, 0:1], 0.0
                )
                nc.scalar.dma_start(
                    wt[:, :ns, :, :], wgf[j // NQ, j % NQ, :, :ns, :, :]
                )
                wg_tiles[j] = wt
            h2e, h2o = conv1(h1d)
            prev = (h2e, h2o, blk)
        h2e, h2o, pblk = prev
        h3e, h3o = conv2(h2e, h2o)
        conv3(h3e, h3o, pblk)

        # ---- fc1 (f-outer) + inline fc2 ----
        cpsum_ctx.close()
        fpsum_ctx = ExitStack()
        fpsum = fpsum_ctx.enter_context(tc.tile_pool(name="fpsum", bufs=2, space="PSUM"))
        f2psum = fpsum_ctx.enter_context(tc.tile_pool(name="f2psum", bufs=1, space="PSUM"))
        f2ps = f2psum.tile([2, BC], F32, tag="ps2", name="ps2")
        for f in range(8):
            fps = fpsum.tile([128, BC], F32, tag="fps", name=f"fps{f}")
            nmm = 0
            for q in range(NQ):
                j = NQ * f + q
                ns = NSQ[q]
                if j in wg_tiles:
                    wt = wg_tiles.pop(j)
                else:
                    # slot reuse (WAR on chunk j-NGATE's matmul reads) paces
                    # these transfers; round-robin all 3 DMA queues (each
                    # sustains ~105GB/s; fc1 consumes ~300GB/s)
                    wt = wgpool.tile([128, QS, 2, 128], BF16, name="wq", tag="wgf")
                    eng = (nc.sync, nc.scalar, nc.gpsimd)[j % 3]
                    eng.dma_start(
                        wt[:, :ns, :, :], wgf[f, q, :, :ns, :, :]
                    )
                for s in range(ns):
                    g = OFF[q] + s
                    for c in range(2):
                        rhs = zres[:, c, g, :] if g < 64 else fbyp[:, c, :]
                        nc.tensor.matmul(
                            fps[:], wt[:, s, c, :], rhs,
                            start=(nmm == 0), stop=(nmm == 129),
                        )
                        nmm += 1
            z2f = z2pool.tile([128, BC], F32R, tag="z2")
            nc.scalar.activation(
                z2f[:], fps[:], RELU, bias=bias_t[:, 10 + f : 11 + f],
            )
            nc.tensor.matmul(
                f2ps[:], wfc2_t[:, f, :], z2f[:],
                start=(f == 0), stop=(f == 7),
            )

        osb = spool.tile([2, BC], F32)
        nc.vector.tensor_scalar_add(osb[:], f2ps[:], bias_t[:2, 18:19])
        nc.sync.dma_start(out[:], osb[:])
        fpsum_ctx.close()

    nc.compile()
    return nc


def _prep_inputs(inputs):
    """Host-side layout prep. Returns per-core input maps."""
    f32 = lambda a: np.ascontiguousarray(np.asarray(a), dtype=np.float32)
    x = f32(inputs["x"])
    Wp = f32(inputs["Wp"])
    W1, W2, W3 = f32(inputs["W1"]), f32(inputs["W2"]), f32(inputs["W3"])
    Wb1, Wb2, Wb3 = f32(inputs["Wb1"]), f32(inputs["Wb2"]), f32(inputs["Wb3"])
    Wfc1, Wfc2 = f32(inputs["Wfc1"]), f32(inputs["Wfc2"])

    xr3 = x.reshape(B, CL, IL)  # [b, c, i]
    xT = np.ascontiguousarray(xr3.transpose(2, 1, 0))  # [i, c, b]
    x0T = np.ascontiguousarray(xr3[:, 0, :].T)  # [i, b]

    tnp = ml_dtypes.bfloat16
    w1n_np = np.zeros((128, 256), np.float32)
    w1n_np[0:64, 0:128] = W1[:, :, 0].T
    w1n_np[64:128, 0:128] = W1[:, :, 1].T
    w1n_np[64:128, 128:256] = W1[:, :, 2].T

    # fc1 weights -> [8 f, 8 chunks, 128 part, <=9 slabs, 2 ci, 128 f]
    C3_OUT = CH3 * L3
    wg = np.ascontiguousarray(
        Wfc1[:, :C3_OUT].reshape(F1, CH3, L3).transpose(2, 1, 0)
        .reshape(L3, 2, 128, F1).transpose(0, 2, 1, 3)
    )  # [l3, 128, ci, F1]
    wbyp = np.ascontiguousarray(
        Wfc1[:, C3_OUT:].T.reshape(2, 128, F1).transpose(1, 0, 2)
    )  # [128, ci, F1]
    wgf_np = np.zeros((8, NQ, 128, QS, 2, 128), np.float32)
    for f in range(8):
        fsl = slice(f * 128, (f + 1) * 128)
        for q in range(NQ):
            for s in range(NSQ[q]):
                g = OFF[q] + s
                if g < 64:
                    wgf_np[f, q, :, s, :, :] = wg[g, :, :, fsl]
                else:
                    wgf_np[f, q, :, s, :, :] = wbyp[:, :, fsl]

    shared = {
        "wstk": np.ascontiguousarray(
            np.concatenate([Wp[:, :, 1].T, Wp[:, :, 0].T], axis=0)
        ).astype(tnp),
        "w1n": w1n_np.astype(tnp),
        "w2": np.ascontiguousarray(W2.transpose(1, 2, 0)).astype(tnp),
        "w3": np.ascontiguousarray(
            W3.transpose(1, 2, 0).reshape(2, 128, 3, CH3).transpose(1, 0, 2, 3)
        ).astype(tnp),
        "wb1": np.ascontiguousarray(Wb1.T),
        "wb2": np.ascontiguousarray(Wb2.T),
        "wb3": np.ascontiguousarray(Wb3.T),
        "wgf": wgf_np.astype(ml_dtypes.bfloat16),
        "wfc2": np.ascontiguousarray(
            Wfc2.T.reshape(8, 128, OUTC).transpose(1, 0, 2)
        ),
    }

    bias_np = np.zeros((128, NBIAS), np.float32)
    bias_np[:64, 0] = f32(inputs["bp"])
    bias_np[64:, 0] = f32(inputs["bp"])
    bias_np[:, 1] = f32(inputs["b1"])
    b2, b3 = f32(inputs["b2"]), f32(inputs["b3"])
    bias_np[:, 2], bias_np[:, 3] = b2[:128], b2[128:]
    bias_np[:, 4], bias_np[:, 5] = b3[:128], b3[128:]
    bias_np[:64, 6] = f32(inputs["bb1"])
    bias_np[:, 7] = f32(inputs["bb2"])
    bb3 = f32(inputs["bb3"])
    bias_np[:, 8], bias_np[:, 9] = bb3[:128], bb3[128:]
    bias_np[:, 10:18] = f32(inputs["bfc1"]).reshape(8, 128).T
    bias_np[:2, 18] = f32(inputs["bfc2"])
    shared["bias"] = bias_np

    in_maps = []
    for core in range(NCORES):
        sl = slice(core * BC, (core + 1) * BC)
        xc = xT[:, :, sl].reshape(IL, CL, NBLK, BB)
        x0b = x0T[:, sl].reshape(IL, NBLK, BB)
        xs_core = np.empty((NBLK, 128, CL, BB), tnp)
        xs_core[:, :64] = xc.transpose(2, 0, 1, 3)
        xs_core[:, 64:] = x0b.transpose(1, 0, 2)[:, :, None, :]
        m = dict(shared)
        m["xs"] = xs_core
        m["x0s"] = np.ascontiguousarray(x0T[:, sl])
        in_maps.append(m)
    return in_maps


_NC_CACHE = {}


def _get_nc():
    if "nc" not in _NC_CACHE:
        _NC_CACHE["nc"] = build_nc()
    return _NC_CACHE["nc"]


def run(inputs, trace=False):
    from concourse.bass_utils import run_bass_kernel_spmd

    nc = _get_nc()
    in_maps = _prep_inputs(inputs)
    res = run_bass_kernel_spmd(
        nc, in_maps, core_ids=list(range(NCORES)), trace=trace
    )
    outs = [np.asarray(r["out"]) for r in res.results]
    full = np.concatenate([o.T for o in outs], axis=0).astype(np.float32)
    return full, res


def kernel(**inputs) -> np.ndarray:
    full, _ = run(inputs, trace=False)
    return full
